# revision 12
# baseline (speedup 1.0000x reference)
"""DTCWT1D Trainium2 kernel (8-core SPMD, batch-parallel).

All three wavelet levels are banded-Toeplitz matmuls on the TensorEngine in
float32r (fast fp32 mode, ~1.6e-4 relerr vs fp64).

Grid convention: G[q, c] = y[pos0 + rowblk*c + q], zero outside [0, L).
 - x grid: 128 rows, pos0 = -128 (col 0 zero, data cols 1..L/128, then zero)
 - level-output grids: 64 rows per filter; conv:
     out[64f+g, CC] = sum_d sum_q W[d][q, 64f+g] * G_in[q, cmap*CC + d]
   cmap = 128//rowblk_in; d in {0,1} (L1) / {0,1,2} (L2, L3).
First/last grid columns contain out-of-range slots; they are computed as their
own width-1 chunks with edge-variant weights whose dead output rows are zero,
so the lo grids stay exactly zero outside [0, L) with full-partition copies.

Device pipeline per sample: GPSIMD DMA-casts x (fp32->fp32r) into the grid;
PE runs matmul chunks (PSUM 8-bank rotation, weight-major groups of 4 so the
stationary operand reload is skipped); ACT evacuates lo-grids with a
fp32->fp32r cast (feeding the next level); DVE evacuates final outputs as
fp32; SYNC DMAs each output tensor once per sample. The host pre-transposes x
into the partition-major grid and inverse-maps the output grids.
"""
import math
import numpy as np

import concourse.bass as bass
import concourse.mybir as mybir
from concourse.bass_utils import run_bass_kernel_spmd

F32 = mybir.dt.float32
F32R = mybir.dt.float32r

N_CORES = 8
POS0_X, POS0_L1, POS0_L2, POS0_L3 = -128, -62, -28, -11

# stage meta: (wbase, n_d, rows_in, pos0_in, pos0_out, mrows)
STAGE_META = [
    ("w1", 2, 128, POS0_X, POS0_L1, 128),
    ("w2a", 3, 64, POS0_L1, POS0_L2, 128),
    ("w2b", 3, 64, POS0_L1, POS0_L2, 64),
    ("w3a", 3, 64, POS0_L2, POS0_L3, 128),
    ("w3b", 3, 64, POS0_L2, POS0_L3, 64),
]


def build_weights(filters, rowblk_in, pos0_in, pos0_out, dmax):
    nf = len(filters)
    Ws = [np.zeros((rowblk_in, 64 * nf), np.float32) for _ in range(dmax + 1)]
    for f, h in enumerate(filters):
        h = np.asarray(h, dtype=np.float32)
        K = len(h)
        pl = (K - 1) // 2
        for g in range(64):
            for k in range(K):
                num = 2 * pos0_out + 2 * g + k - pl - pos0_in
                d, q = divmod(num, rowblk_in)
                assert 0 <= d <= dmax, (f, g, k, d)
                Ws[d][q, 64 * f + g] += h[k]
    return Ws


def edge_variants(Ws, pos0_out, Lout, nc_cols):
    """(W_first, W_last): dead output rows zeroed for grid col 0 / col nc-1."""
    gs = -pos0_out
    ge = Lout - (pos0_out + 64 * (nc_cols - 1))
    assert 0 < gs <= 64 and 0 < ge <= 64
    Wf, Wl = [], []
    for W in Ws:
        M = W.shape[1]
        mask_f = np.ones(M, np.float32)
        mask_l = np.ones(M, np.float32)
        for f in range(M // 64):
            mask_f[f * 64: f * 64 + gs] = 0.0
            mask_l[f * 64 + ge: (f + 1) * 64] = 0.0
        Wf.append(W * mask_f[None, :])
        Wl.append(W * mask_l[None, :])
    return Wf, Wl


def _chunks(nc_cols):
    """Chunk plan. fp32r matmuls need an even moving free-dim, so all widths
    are even. Order matters: the f-edge chunk (masked weights for grid col 0)
    runs first and regular chunks overwrite its cols >= 1; the l-edge chunk
    runs last but only its final column (grid col nc_cols-1) is evacuated.
    The regular cover of [1, nc_cols-1) may extend one col into nc_cols-1
    (even-width padding); the l-chunk's restricted evacuation fixes that col.
    """
    assert nc_cols >= 66 and nc_cols % 2 == 1
    out = [(0, 64, "f")]
    c0 = 1
    while c0 < nc_cols - 1:
        w = min(512, nc_cols - 1 - c0)
        if w % 2:
            w += 1  # forward-extend into the last col; l-chunk re-fixes it
        out.append((c0, w, "r"))
        c0 += w
    out.append((nc_cols - 64, 64, "l"))
    return out


def _wnames():
    names = []
    for (wb, nd, ri, _pi, _po, mr) in STAGE_META:
        for d in range(nd):
            for v in "rfl":
                names.append((f"{wb}_{d}{v}", [ri, mr]))
    return names


def build_program(L, spc):
    NBX = L // 128
    L1o, L2o, L3o = L // 2, L // 4, L // 8
    NC1 = math.ceil((L1o - POS0_L1) / 64)
    NC2 = math.ceil((L2o - POS0_L2) / 64)
    NC3 = math.ceil((L3o - POS0_L3) / 64)
    NCX_T = NBX + 2
    NC1_T = NC1 + 2
    NC2_T = NC2 + 2
    assert 2 * (NC2 - 1) + 2 <= NC1_T - 1
    assert 2 * (NC3 - 1) + 2 <= NC2_T - 1

    nc = bass.Bass()
    x_ext = nc.declare_dram_parameter("x", [spc * 128, NBX], F32, isOutput=False)
    wshapes = _wnames()
    w_ext = {n: nc.declare_dram_parameter(n, s, F32, isOutput=False)
             for n, s in wshapes}
    zz_ext = nc.declare_dram_parameter("zz", [128, 4], F32, isOutput=False)
    o_ext = {
        "hi1": nc.declare_dram_parameter("hi1", [spc, 64, NC1], F32, isOutput=True),
        "hia2": nc.declare_dram_parameter("hia2", [spc, 64, NC2], F32, isOutput=True),
        "hib2": nc.declare_dram_parameter("hib2", [spc, 64, NC2], F32, isOutput=True),
        "lo3": nc.declare_dram_parameter("lo3", [spc, 64, NC3], F32, isOutput=True),
        "hia3": nc.declare_dram_parameter("hia3", [spc, 64, NC3], F32, isOutput=True),
        "hib3": nc.declare_dram_parameter("hib3", [spc, 64, NC3], F32, isOutput=True),
    }

    from contextlib import ExitStack
    es = ExitStack()
    XC = es.enter_context(nc.sbuf_tensor("XC", [128, NCX_T], F32R))
    E1 = es.enter_context(nc.sbuf_tensor("E1", [64, NC1_T], F32R))
    H1 = es.enter_context(nc.sbuf_tensor("H1", [128, NC1], F32))
    E2 = es.enter_context(nc.sbuf_tensor("E2", [64, NC2_T], F32R))
    HA2 = es.enter_context(nc.sbuf_tensor("HA2", [128, NC2], F32))
    T2b = es.enter_context(nc.sbuf_tensor("T2b", [64, NC2], F32))
    T3 = es.enter_context(nc.sbuf_tensor("T3", [128, NC3], F32))
    T3b = es.enter_context(nc.sbuf_tensor("T3b", [64, NC3], F32))
    WT = {n: es.enter_context(nc.sbuf_tensor("s" + n, s, F32R))
          for n, s in wshapes}
    PB = [es.enter_context(nc.psum_tensor(f"pb{i}", [128, 512], F32))
          for i in range(8)]
    dma_x = es.enter_context(nc.semaphore("dma_x"))
    dma_w = es.enter_context(nc.semaphore("dma_w"))
    pe_s = es.enter_context(nc.semaphore("pe_s"))
    act_s = es.enter_context(nc.semaphore("act_s"))
    dve_s = es.enter_context(nc.semaphore("dve_s"))
    out_s = es.enter_context(nc.semaphore("out_s"))

    def rhs_x(c0, w, d):
        return XC[:, c0 + d: c0 + d + w]

    def rhs_e1(c0, w, d):
        a = 2 * c0 + d
        return E1[0:64, a: a + 2 * w - 1: 2]

    def rhs_e2(c0, w, d):
        a = 2 * c0 + d
        return E2[0:64, a: a + 2 * w - 1: 2]

    stages = [
        dict(meta=STAGE_META[0], rhs=rhs_x, chunks=_chunks(NC1), ncols=NC1,
             evacs=[("act", 0, 64, lambda: E1),
                    ("dve", 64, 128, lambda: H1[64:128])]),
        dict(meta=STAGE_META[1], rhs=rhs_e1, chunks=_chunks(NC2), ncols=NC2,
             evacs=[("act", 0, 64, lambda: E2),
                    ("dve", 64, 128, lambda: HA2[64:128])]),
        dict(meta=STAGE_META[2], rhs=rhs_e1, chunks=_chunks(NC2), ncols=NC2,
             evacs=[("act", 0, 64, lambda: T2b)]),
        dict(meta=STAGE_META[3], rhs=rhs_e2, chunks=_chunks(NC3), ncols=NC3,
             evacs=[("dve", 0, 64, lambda: T3[0:64]),
                    ("dve", 64, 128, lambda: T3[64:128])]),
        dict(meta=STAGE_META[4], rhs=rhs_e2, chunks=_chunks(NC3), ncols=NC3,
             evacs=[("act", 0, 64, lambda: T3b)]),
    ]
    outmap = [
        ("hi1", lambda: H1[64:128, 0:NC1], 0),
        ("hia2", lambda: HA2[64:128, 0:NC2], 1),
        ("hib2", lambda: T2b[0:64, 0:NC2], 2),
        ("lo3", lambda: T3[0:64, 0:NC3], 3),
        ("hia3", lambda: T3[64:128, 0:NC3], 3),
        ("hib3", lambda: T3b[0:64, 0:NC3], 4),
    ]

    jobs = []
    for s in range(spc):
        for st in range(len(stages)):
            for ci, (c0, w, var) in enumerate(stages[st]["chunks"]):
                jobs.append((s, st, ci, c0, w, var))

    act_after, dve_after = [], []
    a_cnt = d_cnt = 0
    for (s, st, ci, c0, w, var) in jobs:
        for ev in stages[st]["evacs"]:
            if ev[0] == "act":
                a_cnt += 1
            else:
                d_cnt += 1
        act_after.append(a_cnt)
        dve_after.append(d_cnt)

    stage_end = {}
    for j, (s, st, ci, c0, w, var) in enumerate(jobs):
        stage_end[(s, st)] = j

    XP = 4
    piece_cols = NBX // XP
    N_ZDMA = 4
    N_WLOADS = len(WT) + N_ZDMA

    def wname_for(st, var, d):
        return f"{STAGE_META[st][0]}_{d}{var}"

    block_es = ExitStack()
    block = block_es.enter_context(nc.Block())
    with block_es:
        @block.gpsimd
        def _(g):
            for name, t in WT.items():
                g.dma_start(out=t[:], in_=w_ext[name][:]).then_inc(dma_w, 16)
            with nc.allow_non_contiguous_dma(reason="one-off tiny edge zeros"):
                g.dma_start(out=XC[:, 0:1], in_=zz_ext[:, 0:1]).then_inc(dma_w, 16)
                g.dma_start(out=XC[:, NBX + 1: NCX_T],
                            in_=zz_ext[:, 0: NCX_T - NBX - 1]).then_inc(dma_w, 16)
                g.dma_start(out=E1[0:64, NC1: NC1_T],
                            in_=zz_ext[0:64, 0: NC1_T - NC1]).then_inc(dma_w, 16)
                g.dma_start(out=E2[0:64, NC2: NC2_T],
                            in_=zz_ext[0:64, 0: NC2_T - NC2]).then_inc(dma_w, 16)
            for s in range(spc):
                if s > 0:
                    g.wait_ge(pe_s, stage_end[(s - 1, 0)] + 1)
                for p in range(XP):
                    g.dma_start(
                        out=XC[:, 1 + p * piece_cols: 1 + (p + 1) * piece_cols],
                        in_=x_ext[s * 128:(s + 1) * 128,
                                  p * piece_cols:(p + 1) * piece_cols],
                    ).then_inc(dma_x, 16)

        groups = []
        for j, (s, st, ci, c0, w, var) in enumerate(jobs):
            if ci % 4 == 0:
                groups.append((s, st, []))
            groups[-1][2].append(j)

        @block.tensor
        def _(t):
            t.wait_ge(dma_w, 16 * N_WLOADS)
            for (s, st, js) in groups:
                stg = stages[st]
                j_last = js[-1]
                ci0 = jobs[js[0]][2]
                if st == 0:
                    mx = max(jobs[j][3] + jobs[j][4] for j in js)
                    p = min(XP - 1, max(0, mx - 1) // piece_cols)
                    t.wait_ge(dma_x, 16 * (XP * s + p + 1))
                if j_last >= 8:
                    t.wait_ge(act_s, act_after[j_last - 8])
                    t.wait_ge(dve_s, dve_after[j_last - 8])
                if ci0 == 0:
                    if st in (1, 2):
                        t.wait_ge(act_s, act_after[stage_end[(s, 0)]])
                    elif st in (3, 4):
                        t.wait_ge(act_s, act_after[stage_end[(s, 1)]])
                nd = stg["meta"][1]
                mrows = stg["meta"][5]
                for d in range(nd):
                    for j in js:
                        _s, _st, ci, c0, w, var = jobs[j]
                        bank = PB[j % 8]
                        mm = t.matmul(bank[0:mrows, 0:w],
                                      WT[wname_for(st, var, d)][:],
                                      stg["rhs"](c0, w, d),
                                      start=(d == 0), stop=(d == nd - 1),
                                      skip_group_check=True)
                        if d == nd - 1:
                            mm.then_inc(pe_s, 1)

        @block.scalar
        def _(a):
            for j, (s, st, ci, c0, w, var) in enumerate(jobs):
                stg = stages[st]
                acts = [e for e in stg["evacs"] if e[0] == "act"]
                if not acts:
                    continue
                bank = PB[j % 8]
                a.wait_ge(pe_s, j + 1)
                if ci == 0 and s > 0:
                    for oi, (oname, _f, ost) in enumerate(outmap):
                        if ost == st:
                            a.wait_ge(out_s, 16 * (6 * (s - 1) + oi + 1))
                ec0, ep0, ew = (c0, 0, w) if var != "l" else (c0 + w - 1, w - 1, 1)
                for (eng, r0, r1, dst) in acts:
                    a.copy(dst()[0:64, ec0:ec0 + ew],
                           bank[r0:r1, ep0:ep0 + ew]).then_inc(act_s, 1)

        @block.vector
        def _(v):
            for j, (s, st, ci, c0, w, var) in enumerate(jobs):
                stg = stages[st]
                dves = [e for e in stg["evacs"] if e[0] == "dve"]
                if not dves:
                    continue
                bank = PB[j % 8]
                v.wait_ge(pe_s, j + 1)
                if ci == 0 and s > 0:
                    for oi, (oname, _f, ost) in enumerate(outmap):
                        if ost == st:
                            v.wait_ge(out_s, 16 * (6 * (s - 1) + oi + 1))
                ec0, ep0, ew = (c0, 0, w) if var != "l" else (c0 + w - 1, w - 1, 1)
                for (eng, r0, r1, dst) in dves:
                    v.tensor_copy(dst()[:, ec0:ec0 + ew],
                                  bank[r0:r1, ep0:ep0 + ew]).then_inc(dve_s, 1)

        @block.sync
        def _(sy):
            for s in range(spc):
                for oi, (oname, ap_fn, ost) in enumerate(outmap):
                    jend = stage_end[(s, ost)]
                    sy.wait_ge(act_s, act_after[jend])
                    sy.wait_ge(dve_s, dve_after[jend])
                    sy.dma_start(out=o_ext[oname][s], in_=ap_fn()
                                 ).then_inc(out_s, 16)

    return nc


# ---------------- host side ----------------

def _grid_to_linear(arr, pos0, Lout):
    Bn = arr.shape[0]
    flat = arr.transpose(0, 2, 1).reshape(Bn, -1)
    return flat[:, -pos0:-pos0 + Lout]


_CACHE = {}
LAST_RUN_S = None


def kernel(x, h0o, h1o, h0a, h1a, h0b, h1b):
    x = np.asarray(x)
    B, C, L = x.shape
    assert C == 1 and B % N_CORES == 0
    spc = B // N_CORES
    L1o, L2o, L3o = L // 2, L // 4, L // 8
    NBX = L // 128
    NC1 = math.ceil((L1o - POS0_L1) / 64)
    NC2 = math.ceil((L2o - POS0_L2) / 64)
    NC3 = math.ceil((L3o - POS0_L3) / 64)

    key = (L, spc)
    if key not in _CACHE:
        _CACHE[key] = build_program(L, spc)
    nc = _CACHE[key]

    filt = {
        "w1": ([h0o, h1o], 128, POS0_X, POS0_L1, 1, L1o, NC1),
        "w2a": ([h0a, h1a], 64, POS0_L1, POS0_L2, 2, L2o, NC2),
        "w2b": ([h1b], 64, POS0_L1, POS0_L2, 2, L2o, NC2),
        "w3a": ([h0a, h1a], 64, POS0_L2, POS0_L3, 2, L3o, NC3),
        "w3b": ([h1b], 64, POS0_L2, POS0_L3, 2, L3o, NC3),
    }
    wmap = {}
    for wb, (fl, ri, pi, po, dmax, Lout, ncc) in filt.items():
        Ws = build_weights(fl, ri, pi, po, dmax)
        Wf, Wl = edge_variants(Ws, po, Lout, ncc)
        for d in range(dmax + 1):
            wmap[f"{wb}_{d}r"] = Ws[d]
            wmap[f"{wb}_{d}f"] = Wf[d]
            wmap[f"{wb}_{d}l"] = Wl[d]
    wmap["zz"] = np.zeros((128, 4), np.float32)

    xg = np.ascontiguousarray(x.reshape(B, NBX, 128).transpose(0, 2, 1))

    in_maps = []
    for c in range(N_CORES):
        m = {"x": xg[c * spc:(c + 1) * spc].reshape(spc * 128, NBX)}
        m.update(wmap)
        in_maps.append(m)

    import time as _time
    _t0 = _time.perf_counter()
    res = run_bass_kernel_spmd(nc, in_maps, list(range(N_CORES)))
    global LAST_RUN_S
    LAST_RUN_S = _time.perf_counter() - _t0

    def gather(name, pos0, Lout):
        arr = np.concatenate([res.results[c][name] for c in range(N_CORES)],
                             axis=0)
        return _grid_to_linear(arr, pos0, Lout)

    lo = gather("lo3", POS0_L3, L3o).reshape(B, 1, L3o)
    hi1 = gather("hi1", POS0_L1, L1o).reshape(B, 1, 1, L1o)
    yh1 = np.stack((gather("hia2", POS0_L2, L2o),
                    gather("hib2", POS0_L2, L2o)), axis=1).reshape(B, 2, 1, L2o)
    yh2 = np.stack((gather("hia3", POS0_L3, L3o),
                    gather("hib3", POS0_L3, L3o)), axis=1).reshape(B, 2, 1, L3o)
    return (np.ascontiguousarray(lo, dtype=np.float32),
            np.ascontiguousarray(hi1, dtype=np.float32),
            np.ascontiguousarray(yh1, dtype=np.float32),
            np.ascontiguousarray(yh2, dtype=np.float32))


# revision 13
# speedup vs baseline: 1.1223x; 1.1223x over previous
"""DTCWT1D Trainium2 kernel (8-core SPMD, batch-parallel).

All three wavelet levels are banded-Toeplitz matmuls on the TensorEngine in
float32r (fast fp32 mode, ~1.6e-4 relerr vs fp64).

Grid convention: G[q, c] = y[pos0 + rowblk*c + q], zero outside [0, L).
 - x grid: 128 rows, pos0 = -128 (col 0 zero, data cols 1..L/128, then zero)
 - level-output grids: 64 rows per filter; conv:
     out[64f+g, CC] = sum_d sum_q W[d][q, 64f+g] * G_in[q, cmap*CC + d]
   cmap = 128//rowblk_in; d in {0,1} (L1) / {0,1,2} (L2, L3).
First/last grid columns contain out-of-range slots; they are computed as their
own width-1 chunks with edge-variant weights whose dead output rows are zero,
so the lo grids stay exactly zero outside [0, L) with full-partition copies.

Device pipeline per sample: GPSIMD DMA-casts x (fp32->fp32r) into the grid;
PE runs matmul chunks (PSUM 8-bank rotation, weight-major groups of 4 so the
stationary operand reload is skipped); ACT evacuates lo-grids with a
fp32->fp32r cast (feeding the next level); DVE evacuates final outputs as
fp32; SYNC DMAs each output tensor once per sample. The host pre-transposes x
into the partition-major grid and inverse-maps the output grids.
"""
import math
import numpy as np

import concourse.bass as bass
import concourse.mybir as mybir
from concourse.bass_utils import run_bass_kernel_spmd

F32 = mybir.dt.float32
F32R = mybir.dt.float32r

N_CORES = 8
POS0_X, POS0_L1, POS0_L2, POS0_L3 = -128, -62, -28, -11

# stage meta: (wbase, n_d, rows_in, pos0_in, pos0_out, mrows)
STAGE_META = [
    ("w1", 2, 128, POS0_X, POS0_L1, 128),
    ("w2a", 3, 64, POS0_L1, POS0_L2, 128),
    ("w2b", 3, 64, POS0_L1, POS0_L2, 64),
    ("w3a", 3, 64, POS0_L2, POS0_L3, 128),
    ("w3b", 3, 64, POS0_L2, POS0_L3, 64),
]


def build_weights(filters, rowblk_in, pos0_in, pos0_out, dmax):
    nf = len(filters)
    Ws = [np.zeros((rowblk_in, 64 * nf), np.float32) for _ in range(dmax + 1)]
    for f, h in enumerate(filters):
        h = np.asarray(h, dtype=np.float32)
        K = len(h)
        pl = (K - 1) // 2
        for g in range(64):
            for k in range(K):
                num = 2 * pos0_out + 2 * g + k - pl - pos0_in
                d, q = divmod(num, rowblk_in)
                assert 0 <= d <= dmax, (f, g, k, d)
                Ws[d][q, 64 * f + g] += h[k]
    return Ws


def edge_variants(Ws, pos0_out, Lout, nc_cols):
    """(W_first, W_last): dead output rows zeroed for grid col 0 / col nc-1."""
    gs = -pos0_out
    ge = Lout - (pos0_out + 64 * (nc_cols - 1))
    assert 0 < gs <= 64 and 0 < ge <= 64
    Wf, Wl = [], []
    for W in Ws:
        M = W.shape[1]
        mask_f = np.ones(M, np.float32)
        mask_l = np.ones(M, np.float32)
        for f in range(M // 64):
            mask_f[f * 64: f * 64 + gs] = 0.0
            mask_l[f * 64 + ge: (f + 1) * 64] = 0.0
        Wf.append(W * mask_f[None, :])
        Wl.append(W * mask_l[None, :])
    return Wf, Wl


def _chunks(nc_cols):
    """Chunk plan. fp32r matmuls need an even moving free-dim, so all widths
    are even. Order matters: the f-edge chunk (masked weights for grid col 0)
    runs first and regular chunks overwrite its cols >= 1; the l-edge chunk
    runs last but only its final column (grid col nc_cols-1) is evacuated.
    The regular cover of [1, nc_cols-1) may extend one col into nc_cols-1
    (even-width padding); the l-chunk's restricted evacuation fixes that col.
    """
    assert nc_cols >= 66 and nc_cols % 2 == 1
    out = [(0, 64, "f")]
    c0 = 1
    while c0 < nc_cols - 1:
        w = min(512, nc_cols - 1 - c0)
        if w % 2:
            w += 1  # forward-extend into the last col; l-chunk re-fixes it
        out.append((c0, w, "r"))
        c0 += w
    out.append((nc_cols - 64, 64, "l"))
    return out


def _wnames():
    names = []
    for (wb, nd, ri, _pi, _po, mr) in STAGE_META:
        for d in range(nd):
            for v in "rfl":
                names.append((f"{wb}_{d}{v}", [ri, mr]))
    return names


def build_program(L, spc, repeat=1):
    NBX = L // 128
    L1o, L2o, L3o = L // 2, L // 4, L // 8
    NC1 = math.ceil((L1o - POS0_L1) / 64)
    NC2 = math.ceil((L2o - POS0_L2) / 64)
    NC3 = math.ceil((L3o - POS0_L3) / 64)
    NCX_T = NBX + 2
    NC1_T = NC1 + 2
    NC2_T = NC2 + 2
    assert 2 * (NC2 - 1) + 2 <= NC1_T - 1
    assert 2 * (NC3 - 1) + 2 <= NC2_T - 1

    nc = bass.Bass()
    x_ext = nc.declare_dram_parameter("x", [spc * 128, NBX], F32, isOutput=False)
    wshapes = _wnames()
    w_ext = {n: nc.declare_dram_parameter(n, s, F32, isOutput=False)
             for n, s in wshapes}
    zz_ext = nc.declare_dram_parameter("zz", [128, 4], F32, isOutput=False)
    o_ext = {
        "hi1": nc.declare_dram_parameter("hi1", [spc, 64, NC1], F32, isOutput=True),
        "hia2": nc.declare_dram_parameter("hia2", [spc, 64, NC2], F32, isOutput=True),
        "hib2": nc.declare_dram_parameter("hib2", [spc, 64, NC2], F32, isOutput=True),
        "lo3": nc.declare_dram_parameter("lo3", [spc, 64, NC3], F32, isOutput=True),
        "hia3": nc.declare_dram_parameter("hia3", [spc, 64, NC3], F32, isOutput=True),
        "hib3": nc.declare_dram_parameter("hib3", [spc, 64, NC3], F32, isOutput=True),
    }

    from contextlib import ExitStack
    es = ExitStack()
    XC = es.enter_context(nc.sbuf_tensor("XC", [128, NCX_T], F32R))
    E1 = es.enter_context(nc.sbuf_tensor("E1", [64, NC1_T], F32R))
    H1 = es.enter_context(nc.sbuf_tensor("H1", [128, NC1], F32))
    E2 = es.enter_context(nc.sbuf_tensor("E2", [64, NC2_T], F32R))
    HA2 = es.enter_context(nc.sbuf_tensor("HA2", [128, NC2], F32))
    T2b = es.enter_context(nc.sbuf_tensor("T2b", [64, NC2], F32))
    T3 = es.enter_context(nc.sbuf_tensor("T3", [128, NC3], F32))
    T3b = es.enter_context(nc.sbuf_tensor("T3b", [64, NC3], F32))
    WT = {n: es.enter_context(nc.sbuf_tensor("s" + n, s, F32R))
          for n, s in wshapes}
    PB = [es.enter_context(nc.psum_tensor(f"pb{i}", [128, 512], F32))
          for i in range(8)]
    dma_x = es.enter_context(nc.semaphore("dma_x"))
    dma_w = es.enter_context(nc.semaphore("dma_w"))
    pe_s = es.enter_context(nc.semaphore("pe_s"))
    act_s = es.enter_context(nc.semaphore("act_s"))
    dve_s = es.enter_context(nc.semaphore("dve_s"))
    out_s = es.enter_context(nc.semaphore("out_s"))

    def rhs_x(c0, w, d):
        return XC[:, c0 + d: c0 + d + w]

    def rhs_e1(c0, w, d):
        a = 2 * c0 + d
        return E1[0:64, a: a + 2 * w - 1: 2]

    def rhs_e2(c0, w, d):
        a = 2 * c0 + d
        return E2[0:64, a: a + 2 * w - 1: 2]

    stages = [
        dict(meta=STAGE_META[0], rhs=rhs_x, chunks=_chunks(NC1), ncols=NC1,
             evacs=[("act", 0, 64, lambda: E1),
                    ("dve", 64, 128, lambda: H1[64:128])]),
        dict(meta=STAGE_META[1], rhs=rhs_e1, chunks=_chunks(NC2), ncols=NC2,
             evacs=[("act", 0, 64, lambda: E2),
                    ("dve", 64, 128, lambda: HA2[64:128])]),
        dict(meta=STAGE_META[2], rhs=rhs_e1, chunks=_chunks(NC2), ncols=NC2,
             evacs=[("act", 0, 64, lambda: T2b)]),
        dict(meta=STAGE_META[3], rhs=rhs_e2, chunks=_chunks(NC3), ncols=NC3,
             evacs=[("dve", 0, 64, lambda: T3[0:64]),
                    ("dve", 64, 128, lambda: T3[64:128])]),
        dict(meta=STAGE_META[4], rhs=rhs_e2, chunks=_chunks(NC3), ncols=NC3,
             evacs=[("act", 0, 64, lambda: T3b)]),
    ]
    outmap = [
        ("hi1", lambda: H1[64:128, 0:NC1], 0),
        ("hia2", lambda: HA2[64:128, 0:NC2], 1),
        ("hib2", lambda: T2b[0:64, 0:NC2], 2),
        ("lo3", lambda: T3[0:64, 0:NC3], 3),
        ("hia3", lambda: T3[64:128, 0:NC3], 3),
        ("hib3", lambda: T3b[0:64, 0:NC3], 4),
    ]

    jobs = []
    for s in range(spc * repeat):
        for st in range(len(stages)):
            for ci, (c0, w, var) in enumerate(stages[st]["chunks"]):
                jobs.append((s, st, ci, c0, w, var))

    act_after, dve_after = [], []
    a_cnt = d_cnt = 0
    for (s, st, ci, c0, w, var) in jobs:
        for ev in stages[st]["evacs"]:
            if ev[0] == "act":
                a_cnt += 1
            else:
                d_cnt += 1
        act_after.append(a_cnt)
        dve_after.append(d_cnt)

    stage_end = {}
    for j, (s, st, ci, c0, w, var) in enumerate(jobs):
        stage_end[(s, st)] = j

    XP = 4
    piece_cols = NBX // XP
    N_ZDMA = 4
    N_WLOADS = len(WT) + N_ZDMA

    def wname_for(st, var, d):
        return f"{STAGE_META[st][0]}_{d}{var}"

    block_es = ExitStack()
    block = block_es.enter_context(nc.Block())
    with block_es:
        @block.gpsimd
        def _(g):
            for name, t in WT.items():
                g.dma_start(out=t[:], in_=w_ext[name][:]).then_inc(dma_w, 16)
            with nc.allow_non_contiguous_dma(reason="one-off tiny edge zeros"):
                g.dma_start(out=XC[:, 0:1], in_=zz_ext[:, 0:1]).then_inc(dma_w, 16)
                g.dma_start(out=XC[:, NBX + 1: NCX_T],
                            in_=zz_ext[:, 0: NCX_T - NBX - 1]).then_inc(dma_w, 16)
                g.dma_start(out=E1[0:64, NC1: NC1_T],
                            in_=zz_ext[0:64, 0: NC1_T - NC1]).then_inc(dma_w, 16)
                g.dma_start(out=E2[0:64, NC2: NC2_T],
                            in_=zz_ext[0:64, 0: NC2_T - NC2]).then_inc(dma_w, 16)
            for s in range(spc * repeat):
                sm = s % spc
                if s > 0:
                    g.wait_ge(pe_s, stage_end[(s - 1, 0)] + 1)
                for p in range(XP):
                    g.dma_start(
                        out=XC[:, 1 + p * piece_cols: 1 + (p + 1) * piece_cols],
                        in_=x_ext[sm * 128:(sm + 1) * 128,
                                  p * piece_cols:(p + 1) * piece_cols],
                    ).then_inc(dma_x, 16)

        groups = []
        for j, (s, st, ci, c0, w, var) in enumerate(jobs):
            if ci % 4 == 0:
                groups.append((s, st, []))
            groups[-1][2].append(j)

        @block.tensor
        def _(t):
            t.wait_ge(dma_w, 16 * N_WLOADS)
            for (s, st, js) in groups:
                stg = stages[st]
                j_last = js[-1]
                ci0 = jobs[js[0]][2]
                if st == 0:
                    mx = max(jobs[j][3] + jobs[j][4] for j in js)
                    p = min(XP - 1, max(0, mx - 1) // piece_cols)
                    t.wait_ge(dma_x, 16 * (XP * s + p + 1))
                if j_last >= 8:
                    t.wait_ge(act_s, act_after[j_last - 8])
                    t.wait_ge(dve_s, dve_after[j_last - 8])
                if ci0 == 0:
                    if st in (1, 2):
                        t.wait_ge(act_s, act_after[stage_end[(s, 0)]])
                    elif st in (3, 4):
                        t.wait_ge(act_s, act_after[stage_end[(s, 1)]])
                nd = stg["meta"][1]
                mrows = stg["meta"][5]
                for d in range(nd):
                    for j in js:
                        _s, _st, ci, c0, w, var = jobs[j]
                        bank = PB[j % 8]
                        mm = t.matmul(bank[0:mrows, 0:w],
                                      WT[wname_for(st, var, d)][:],
                                      stg["rhs"](c0, w, d),
                                      start=(d == 0), stop=(d == nd - 1),
                                      skip_group_check=True)
                        if d == nd - 1:
                            mm.then_inc(pe_s, 1)

        @block.scalar
        def _(a):
            for j, (s, st, ci, c0, w, var) in enumerate(jobs):
                stg = stages[st]
                acts = [e for e in stg["evacs"] if e[0] == "act"]
                if not acts:
                    continue
                bank = PB[j % 8]
                a.wait_ge(pe_s, j + 1)
                if ci == 0 and s > 0:
                    for oi, (oname, _f, ost) in enumerate(outmap):
                        if ost == st:
                            a.wait_ge(out_s, 16 * (6 * (s - 1) + oi + 1))
                ec0, ep0, ew = (c0, 0, w) if var != "l" else (c0 + w - 1, w - 1, 1)
                for (eng, r0, r1, dst) in acts:
                    a.copy(dst()[0:64, ec0:ec0 + ew],
                           bank[r0:r1, ep0:ep0 + ew]).then_inc(act_s, 1)

        @block.vector
        def _(v):
            for j, (s, st, ci, c0, w, var) in enumerate(jobs):
                stg = stages[st]
                dves = [e for e in stg["evacs"] if e[0] == "dve"]
                if not dves:
                    continue
                bank = PB[j % 8]
                v.wait_ge(pe_s, j + 1)
                if ci == 0 and s > 0:
                    for oi, (oname, _f, ost) in enumerate(outmap):
                        if ost == st:
                            v.wait_ge(out_s, 16 * (6 * (s - 1) + oi + 1))
                ec0, ep0, ew = (c0, 0, w) if var != "l" else (c0 + w - 1, w - 1, 1)
                for (eng, r0, r1, dst) in dves:
                    v.tensor_copy(dst()[:, ec0:ec0 + ew],
                                  bank[r0:r1, ep0:ep0 + ew]).then_inc(dve_s, 1)

        @block.sync
        def _(sy):
            for s in range(spc * repeat):
                for oi, (oname, ap_fn, ost) in enumerate(outmap):
                    jend = stage_end[(s, ost)]
                    sy.wait_ge(act_s, act_after[jend])
                    sy.wait_ge(dve_s, dve_after[jend])
                    sy.dma_start(out=o_ext[oname][s % spc], in_=ap_fn()
                                 ).then_inc(out_s, 16)

    return nc


# ---------------- host side ----------------

def _grid_to_linear(arr, pos0, Lout):
    Bn = arr.shape[0]
    flat = arr.transpose(0, 2, 1).reshape(Bn, -1)
    return flat[:, -pos0:-pos0 + Lout]


_CACHE = {}
LAST_RUN_S = None


def kernel(x, h0o, h1o, h0a, h1a, h0b, h1b):
    x = np.asarray(x)
    B, C, L = x.shape
    assert C == 1 and B % N_CORES == 0
    spc = B // N_CORES
    L1o, L2o, L3o = L // 2, L // 4, L // 8
    NBX = L // 128
    NC1 = math.ceil((L1o - POS0_L1) / 64)
    NC2 = math.ceil((L2o - POS0_L2) / 64)
    NC3 = math.ceil((L3o - POS0_L3) / 64)

    key = (L, spc)
    if key not in _CACHE:
        _CACHE[key] = build_program(L, spc)
    nc = _CACHE[key]

    filt = {
        "w1": ([h0o, h1o], 128, POS0_X, POS0_L1, 1, L1o, NC1),
        "w2a": ([h0a, h1a], 64, POS0_L1, POS0_L2, 2, L2o, NC2),
        "w2b": ([h1b], 64, POS0_L1, POS0_L2, 2, L2o, NC2),
        "w3a": ([h0a, h1a], 64, POS0_L2, POS0_L3, 2, L3o, NC3),
        "w3b": ([h1b], 64, POS0_L2, POS0_L3, 2, L3o, NC3),
    }
    wmap = {}
    for wb, (fl, ri, pi, po, dmax, Lout, ncc) in filt.items():
        Ws = build_weights(fl, ri, pi, po, dmax)
        Wf, Wl = edge_variants(Ws, po, Lout, ncc)
        for d in range(dmax + 1):
            wmap[f"{wb}_{d}r"] = Ws[d]
            wmap[f"{wb}_{d}f"] = Wf[d]
            wmap[f"{wb}_{d}l"] = Wl[d]
    wmap["zz"] = np.zeros((128, 4), np.float32)

    xg = np.ascontiguousarray(x.reshape(B, NBX, 128).transpose(0, 2, 1))

    in_maps = []
    for c in range(N_CORES):
        m = {"x": xg[c * spc:(c + 1) * spc].reshape(spc * 128, NBX)}
        m.update(wmap)
        in_maps.append(m)

    import time as _time
    _t0 = _time.perf_counter()
    res = run_bass_kernel_spmd(nc, in_maps, list(range(N_CORES)))
    global LAST_RUN_S
    LAST_RUN_S = _time.perf_counter() - _t0

    def gather(name, pos0, Lout):
        arr = np.concatenate([res.results[c][name] for c in range(N_CORES)],
                             axis=0)
        return _grid_to_linear(arr, pos0, Lout)

    lo = gather("lo3", POS0_L3, L3o).reshape(B, 1, L3o)
    hi1 = gather("hi1", POS0_L1, L1o).reshape(B, 1, 1, L1o)
    yh1 = np.stack((gather("hia2", POS0_L2, L2o),
                    gather("hib2", POS0_L2, L2o)), axis=1).reshape(B, 2, 1, L2o)
    yh2 = np.stack((gather("hia3", POS0_L3, L3o),
                    gather("hib3", POS0_L3, L3o)), axis=1).reshape(B, 2, 1, L3o)
    return (np.ascontiguousarray(lo, dtype=np.float32),
            np.ascontiguousarray(hi1, dtype=np.float32),
            np.ascontiguousarray(yh1, dtype=np.float32),
            np.ascontiguousarray(yh2, dtype=np.float32))
